# revision 23
# baseline (speedup 1.0000x reference)
"""DBSN pretrain loss on 8 Trainium2 NeuronCores.

Pure data parallel over batch (B=8) -> one image per core. Per pixel
(symmetric 3x3 Y=sigma_y, N=sigma_n, M=sigma_mu; d = target - mu):

    t1 = 0.5 * d^T adj(Y) d / det(Y)
    t2 = 0.5 * ln det(N)
    t3 = 0.5 * tr(adj(N) M) / det(N)

v9 design (v6 algorithm, reshaped for minimal DVE instruction count and
1-block-lag pipelining; GpSimd is left idle on purpose -- its SBUF port
is physically shared with the DVE, so Pool offload slows the bottleneck
engine):
  - True-cofactor plane algebra: CF = M1 - M2 in cofactor order
    [C22,C02,C12,C01,C11,C00]; dets use all-[+1] PE weights; the 2x
    off-diagonal factors ride the host-packed sigma_mu planes (u-chain)
    and a 2I PE weight block (q-chain).
  - S-plane order [c,f,i,e,a,b,c] (c duplicated) lets all 9 non-square
    products batch into 4 wide DVE instructions covering BOTH matrices,
    and the det W-products into 1.  DVE: 11 tensor_tensor + 1 stt per
    block vs ~19+ in v6.
  - dmu planes lead the packed input so d = t - m issues as soon as the
    block DMA starts streaming.
  - Cross-engine deps are all >= 1 block old on the DVE critical path.

Per-partition stats out [128, 4]: col0 = sum(t1 + t3), col1 = sum(ln detN).
Host: loss = (c0 + 0.5*c1) / n_pixels.
"""

import sys

if "/opt/trn_rl_repo" not in sys.path:
    sys.path.insert(0, "/opt/trn_rl_repo")

from contextlib import ExitStack

import numpy as np

import concourse.bass as bass  # noqa: F401
import concourse.tile as tile
from concourse import bacc, mybir
from concourse.bass_utils import run_bass_kernel_spmd

f32 = mybir.dt.float32
bf16 = mybir.dt.bfloat16
AF = mybir.ActivationFunctionType
OP = mybir.AluOpType
AX = mybir.AxisListType

B = 8

# host-side plane orders (flat9 = 3*row+col of the symmetric 3x3)
# S-plane order [c,f,i,e,a,b,c] = S02,S12,S22,S11,S00,S01,S02
SIDX = [2, 5, 8, 4, 0, 1, 2]
# sigma_mu planes paired with CF order [C22,C02,C12,C01,C11,C00]
MIDX = [8, 2, 5, 1, 4, 0]
MW = np.array([1.0, 2.0, 2.0, 2.0, 1.0, 1.0], np.float32)
# d planes [d2,d1,d0] (target then mu)
DIDX = [2, 1, 0]

# Keep all activation funcs resolved to one table set (avoids reloads).
_orig_get_tables = None


def _patch_act_tables():
    global _orig_get_tables
    from concourse import bacc as _bacc

    if _orig_get_tables is not None:
        return
    _orig_get_tables = _bacc.get_activation_tables

    def patched(arch):
        tables = dict(_orig_get_tables(arch))
        names = list(tables)
        want = "natural_log_exp_and_others"
        if want in tables:
            need = {AF.Square, AF.Ln, AF.Exp, AF.Copy, AF.Identity}
            if need <= tables[want]:
                return {
                    n: (tables[n] if n == want else set()) for n in names
                }
        return tables

    _bacc.get_activation_tables = patched


def build(nblocks=4, ncols=512):
    F = ncols
    _patch_act_tables()
    nc = bacc.Bacc("TRN2", target_bir_lowering=False, debug=False)

    # packed input: dmu(6) and [SY(7) | SN(7)] planes per block
    dmu_d = nc.dram_tensor("dmu", [nblocks, 128, 6 * F], bf16,
                           kind="ExternalInput").ap()
    s_d = nc.dram_tensor("sig", [nblocks, 128, 14 * F], bf16,
                         kind="ExternalInput").ap()
    sm_d = nc.dram_tensor("smp", [nblocks, 128, 6 * F], bf16,
                          kind="ExternalInput").ap()
    id_d = nc.dram_tensor("ident", [128, 256], bf16,
                          kind="ExternalInput").ap()
    out_d = nc.dram_tensor("out", [128, 4], f32, kind="ExternalOutput").ap()

    with tile.TileContext(nc) as tc, ExitStack() as ctx:
        inp = ctx.enter_context(tc.tile_pool(name="inp", bufs=2))
        wk = ctx.enter_context(tc.tile_pool(name="wk", bufs=2))
        one = ctx.enter_context(tc.tile_pool(name="one", bufs=1))
        psum = ctx.enter_context(tc.tile_pool(name="psum", bufs=2,
                                              space="PSUM"))

        pew = one.tile([128, 256], bf16, name="pew", tag="pew")
        W1 = pew[:, 0:128]    # I
        W2 = pew[:, 128:256]  # 2I

        NE = nblocks
        zs = one.tile([128, NE], f32, name="zs", tag="zs")    # sum t1+t3
        t2s = one.tile([128, NE], f32, name="t2s", tag="t2s")  # sum ln detN
        out_t = one.tile([128, 4], f32, name="out_t", tag="out_t")
        zjunk = one.tile([128, 2 * F], bf16, name="zjunk", tag="zjunk")

        def bc(view, shape):
            return view.to_broadcast(shape)

        prev = None  # (det2, q2, LL, rr, ecol) of previous block

        for i in range(nblocks):
            in_m = inp.tile([128, 6 * F], bf16, name="in_m", tag="in_m")
            nc.sync.dma_start(out=in_m[:], in_=dmu_d[i])
            in_s = inp.tile([128, 14 * F], bf16, name="in_s", tag="in_s")
            nc.scalar.dma_start(out=in_s[:, 0:7 * F],
                                in_=s_d[i].rearrange(
                                    "p (h n) -> p h n", h=2)[:, 0, :])
            nc.gpsimd.dma_start(out=in_s[:, 7 * F:14 * F],
                                in_=s_d[i].rearrange(
                                    "p (h n) -> p h n", h=2)[:, 1, :])
            VV = inp.tile([128, 12 * F], bf16, name="vv", tag="vv")
            nc.sync.dma_start(out=VV[:, 6 * F:12 * F], in_=sm_d[i])
            if i == 0:
                nc.sync.dma_start(out=pew, in_=id_d)

            # S view: [p, g(Y|N), k(7 planes), n]
            Sv = in_s.rearrange("p (g k n) -> p g k n", g=2, k=7)
            M12 = wk.tile([128, 24 * F], bf16, name="m12", tag="m12")
            Mv = M12.rearrange("p (g s n) -> p g s n", g=2, s=12)
            CF = wk.tile([128, 12 * F], bf16, name="cf", tag="cf")
            CFv = CF.rearrange("p (g s n) -> p g s n", g=2, s=6)
            W = wk.tile([128, 6 * F], bf16, name="w", tag="w")
            Wv = W.rearrange("p (g s n) -> p g s n", g=2, s=3)
            D3 = wk.tile([128, 3 * F], bf16, name="d3", tag="d3")
            D3k = D3.rearrange("p (k n) -> p k n", k=3)
            VVs = VV[:, 0:6 * F].rearrange("p (s n) -> p s n", s=6)
            QU = wk.tile([128, 12 * F], bf16, name="qu", tag="qu")

            # ---- DVE ----
            # d = t - m  (planes [d2,d1,d0]); dmu arrives first in the DMA
            nc.vector.tensor_tensor(
                D3[:], in_m[:, 0:3 * F], in_m[:, 3 * F:6 * F], OP.subtract)
            # Pa: b*[f,c] -> M1[1]=bf, M1[2]=bc
            nc.vector.tensor_tensor(
                Mv[:, :, 1:3, :], bc(Sv[:, :, 5:6, :], (128, 2, 2, F)),
                Sv[:, :, 1::-1, :], OP.mult)
            # Pb: [b,c]*[i,e] -> M2[3]=bi (slot 9), M2[1]=ce (slot 7)
            nc.vector.tensor_tensor(
                Mv[:, :, 9:6:-2, :], Sv[:, :, 5::-5, :],
                Sv[:, :, 2:4, :], OP.mult)
            # Pc: a*[e,i,f] -> M1[0]=ae, M1[4]=ai, M2[2]=af (slots 0,4,8)
            nc.vector.tensor_tensor(
                Mv[:, :, 0:12:4, :], bc(Sv[:, :, 4:5, :], (128, 2, 3, F)),
                Sv[:, :, 3:0:-1, :], OP.mult)
            # Pde: [c,e]*[f,i] -> M1[3]=cf, M1[5]=ei (slots 3,5)
            nc.vector.tensor_tensor(
                Mv[:, :, 3:7:2, :], Sv[:, :, 0:4:3, :],
                Sv[:, :, 1:3, :], OP.mult)
            # ---- ACT (emitted before their DVE consumers CF/QU) ----
            # M2 squares: [c^2,f^2] -> slots 10,11 ; b^2 -> slot 6
            nc.scalar.activation(Mv[:, :, 10:12, :], Sv[:, :, 0:2, :],
                                 AF.Square)
            nc.scalar.activation(Mv[:, :, 6:7, :], Sv[:, :, 5:6, :],
                                 AF.Square)
            # D6 squares: (d2,d1) -> VV slots 0,4 ; d0 -> slot 5
            nc.scalar.activation(VVs[:, 0:5:4, :], D3k[:, 0:2, :],
                                 AF.Square)
            nc.scalar.activation(VVs[:, 5:6, :], D3k[:, 2:3, :],
                                 AF.Square)

            # D6: d0*[d1,d2] -> VV slots 3,1 ; d1*d2 -> slot 2
            nc.vector.tensor_tensor(
                VVs[:, 3:0:-2, :], bc(D3k[:, 2:3, :], (128, 2, F)),
                D3k[:, 1::-1, :], OP.mult)
            nc.vector.tensor_tensor(
                VVs[:, 2:3, :], D3k[:, 1:2, :], D3k[:, 0:1, :], OP.mult)
            # CF = M1 - M2
            nc.vector.tensor_tensor(
                CF[:], Mv[:, :, 0:6, :], Mv[:, :, 6:12, :], OP.subtract)
            # W: [C00,C01,C02]*[a,b,c] (CF slots 5,3,1 ; S slots 4,5,6)
            nc.vector.tensor_tensor(
                Wv[:, :, 0:3, :], CFv[:, :, 5:0:-2, :],
                Sv[:, :, 4:7, :], OP.mult)
            # QU: [Q6|U6] = CF o [D6|SM']
            nc.vector.tensor_tensor(QU[:], CF[:], VV[:], OP.mult)

            # ---- previous block's z (only DVE-consumed tail is deferred) --
            if prev is not None:
                emit_z(nc, prev, zs, zjunk, F)

            det2 = psum.tile([128, 2 * F], f32, name="det2", tag="det2")
            q2 = psum.tile([128, 2 * F], f32, name="q2", tag="q2")
            LL = wk.tile([128, 2 * F], f32, name="ll", tag="ll")
            rr = wk.tile([128, 2 * F], f32, name="rr", tag="rr")

            # PE: det chains (weights I), then u/q chains (same block)
            for g in range(2):
                for j in range(3):
                    nc.tensor.matmul(det2[:, g * F:(g + 1) * F], W1,
                                     Wv[:, g, j, :],
                                     start=(j == 0), stop=(j == 2))
            QUv = QU.rearrange("p (g s n) -> p g s n", g=2, s=6)
            for j in range(6):
                nc.tensor.matmul(q2[:, F:2 * F], W1, QUv[:, 1, j, :],
                                 start=(j == 0), stop=(j == 5))
            # q chain: I on diag slots (0,4,5), 2I on off-diag (1,2,3)
            qorder = [(0, W1, True, False), (4, W1, False, False),
                      (5, W1, False, False), (1, W2, False, False),
                      (2, W2, False, False), (3, W2, False, True)]
            for s, wgt, st, sp in qorder:
                nc.tensor.matmul(q2[:, 0:F], wgt, QUv[:, 0, s, :],
                                 start=st, stop=sp)
            # ACT: logs + reciprocals (same block; ACT has slack)
            nc.scalar.activation(LL[:, 0:F], det2[:, 0:F], AF.Ln)
            nc.scalar.activation(LL[:, F:2 * F], det2[:, F:2 * F], AF.Ln,
                                 accum_out=t2s[:, i:i + 1])
            nc.scalar.activation(rr[:], LL[:], AF.Exp, scale=-1.0)

            prev = (q2, rr, i)

        emit_z(nc, prev, zs, zjunk, F)
        nc.vector.reduce_sum(out_t[:, 0:1], zs[:], axis=AX.X)
        nc.vector.reduce_sum(out_t[:, 1:2], t2s[:], axis=AX.X)
        nc.vector.reduce_sum(out_t[:, 2:3], t2s[:, 0:1], axis=AX.X)
        nc.vector.reduce_sum(out_t[:, 3:4], zs[:, 0:1], axis=AX.X)
        nc.sync.dma_start(out=out_d, in_=out_t[:])

    nc.compile()
    return nc


def emit_z(nc, prev, zs, zjunk, F):
    """z = (q2 * 0.5) * (1/det), accumulated -> sum(t1)+sum(t3)."""
    q2, rr, ecol = prev
    nc.vector.scalar_tensor_tensor(
        zjunk[:], q2[:], 0.5, rr[:], OP.mult, OP.mult,
        accum_out=zs[:, ecol:ecol + 1])


_CACHE = {}


def get_nc(nblocks=4, ncols=512):
    key = (nblocks, ncols)
    if key not in _CACHE:
        _CACHE[key] = build(nblocks, ncols)
    return _CACHE[key]


def make_pew():
    import ml_dtypes

    eye = np.eye(128, dtype=np.float32)
    return np.concatenate([eye, 2.0 * eye], axis=1).astype(ml_dtypes.bfloat16)


def make_in_maps(target, mu, sigma_mu, sigma_n, sigma_y):
    import ml_dtypes

    bf = ml_dtypes.bfloat16
    Bb, C, M, N = target.shape
    nb = M // 128
    F = N
    pew = make_pew()
    in_maps = []
    for b in range(Bb):
        sy = np.asarray(sigma_y[b], np.float32).reshape(M * N, 9)
        sn = np.asarray(sigma_n[b], np.float32).reshape(M * N, 9)
        sm = np.asarray(sigma_mu[b], np.float32).reshape(M * N, 9)
        dmu = np.empty((6, M, N), np.float32)
        dmu[0:3] = np.asarray(target[b], np.float32)[DIDX]
        dmu[3:6] = np.asarray(mu[b], np.float32)[DIDX]
        dmu_p = np.ascontiguousarray(
            dmu.reshape(6, nb, 128, F).transpose(1, 2, 0, 3)
            .reshape(nb, 128, 6 * F).astype(bf))
        sig = np.empty((14, M, N), np.float32)
        sig[0:7] = sy[:, SIDX].T.reshape(7, M, N)
        sig[7:14] = sn[:, SIDX].T.reshape(7, M, N)
        sig_p = np.ascontiguousarray(
            sig.reshape(14, nb, 128, F).transpose(1, 2, 0, 3)
            .reshape(nb, 128, 14 * F).astype(bf))
        smp = (sm[:, MIDX] * MW).T.reshape(6, M, N)
        smp = np.ascontiguousarray(
            smp.reshape(6, nb, 128, F).transpose(1, 2, 0, 3)
            .reshape(nb, 128, 6 * F).astype(bf))
        in_maps.append({"dmu": dmu_p, "sig": sig_p, "smp": smp,
                        "ident": pew})
    return in_maps


def combine(results, n_pixels):
    zsum = 0.0
    t2 = 0.0
    for r in results:
        o = np.asarray(r["out"], dtype=np.float64)
        zsum += o[:, 0].sum()
        t2 += o[:, 1].sum()
    loss = (zsum + 0.5 * t2) / n_pixels
    return np.float32(loss)


def kernel(target, mu, sigma_mu, sigma_n, sigma_y):
    target = np.asarray(target)
    nb = target.shape[2] // 128
    nc = get_nc(nb, target.shape[3])
    in_maps = make_in_maps(target, mu, sigma_mu, sigma_n, sigma_y)
    res = run_bass_kernel_spmd(nc, in_maps, list(range(len(in_maps))))
    n_pixels = target.shape[0] * target.shape[2] * target.shape[3]
    return combine(res.results, n_pixels)


def run_traced(target, mu, sigma_mu, sigma_n, sigma_y, **trace_kwargs):
    target = np.asarray(target)
    nb = target.shape[2] // 128
    nc = get_nc(nb, target.shape[3])
    in_maps = make_in_maps(target, mu, sigma_mu, sigma_n, sigma_y)
    res = run_bass_kernel_spmd(
        nc, in_maps, list(range(len(in_maps))), trace=True, **trace_kwargs)
    n_pixels = target.shape[0] * target.shape[2] * target.shape[3]
    return combine(res.results, n_pixels), res


# revision 24
# speedup vs baseline: 1.0093x; 1.0093x over previous
"""DBSN pretrain loss on 8 Trainium2 NeuronCores.

Pure data parallel over batch (B=8) -> one image per core. Per pixel
(symmetric 3x3 Y=sigma_y, N=sigma_n, M=sigma_mu; d = target - mu):

    t1 = 0.5 * d^T adj(Y) d / det(Y)
    t2 = 0.5 * ln det(N)
    t3 = 0.5 * tr(adj(N) M) / det(N)

v9 design (v6 algorithm, reshaped for minimal DVE instruction count and
1-block-lag pipelining; GpSimd is left idle on purpose -- its SBUF port
is physically shared with the DVE, so Pool offload slows the bottleneck
engine):
  - True-cofactor plane algebra: CF = M1 - M2 in cofactor order
    [C22,C02,C12,C01,C11,C00]; dets use all-[+1] PE weights; the 2x
    off-diagonal factors ride the host-packed sigma_mu planes (u-chain)
    and a 2I PE weight block (q-chain).
  - S-plane order [c,f,i,e,a,b,c] (c duplicated) lets all 9 non-square
    products batch into 4 wide DVE instructions covering BOTH matrices,
    and the det W-products into 1.  DVE: 11 tensor_tensor + 1 stt per
    block vs ~19+ in v6.
  - dmu planes lead the packed input so d = t - m issues as soon as the
    block DMA starts streaming.
  - Cross-engine deps are all >= 1 block old on the DVE critical path.

Per-partition stats out [128, 4]: col0 = sum(t1 + t3), col1 = sum(ln detN).
Host: loss = (c0 + 0.5*c1) / n_pixels.
"""

import sys

if "/opt/trn_rl_repo" not in sys.path:
    sys.path.insert(0, "/opt/trn_rl_repo")

from contextlib import ExitStack

import numpy as np

import concourse.bass as bass  # noqa: F401
import concourse.tile as tile
from concourse import bacc, mybir
from concourse.bass_utils import run_bass_kernel_spmd

f32 = mybir.dt.float32
bf16 = mybir.dt.bfloat16
AF = mybir.ActivationFunctionType
OP = mybir.AluOpType
AX = mybir.AxisListType

B = 8

# host-side plane orders (flat9 = 3*row+col of the symmetric 3x3)
# S-plane order [c,f,i,e,a,b,c] = S02,S12,S22,S11,S00,S01,S02
SIDX = [2, 5, 8, 4, 0, 1, 2]
# sigma_mu planes paired with CF order [C22,C02,C12,C01,C11,C00]
MIDX = [8, 2, 5, 1, 4, 0]
MW = np.array([1.0, 2.0, 2.0, 2.0, 1.0, 1.0], np.float32)
# d planes [d2,d1,d0] (target then mu)
DIDX = [2, 1, 0]

# Keep all activation funcs resolved to one table set (avoids reloads).
_orig_get_tables = None


def _patch_act_tables():
    global _orig_get_tables
    from concourse import bacc as _bacc

    if _orig_get_tables is not None:
        return
    _orig_get_tables = _bacc.get_activation_tables

    def patched(arch):
        tables = dict(_orig_get_tables(arch))
        names = list(tables)
        want = "natural_log_exp_and_others"
        if want in tables:
            need = {AF.Square, AF.Ln, AF.Exp, AF.Copy, AF.Identity}
            if need <= tables[want]:
                return {
                    n: (tables[n] if n == want else set()) for n in names
                }
        return tables

    _bacc.get_activation_tables = patched


def build(nblocks=4, ncols=512):
    F = ncols
    _patch_act_tables()
    nc = bacc.Bacc("TRN2", target_bir_lowering=False, debug=False)

    # packed input: dmu(6) and [SY(7) | SN(7)] planes per block
    dmu_d = nc.dram_tensor("dmu", [nblocks, 128, 6 * F], bf16,
                           kind="ExternalInput").ap()
    s_d = nc.dram_tensor("sig", [nblocks, 128, 14 * F], bf16,
                         kind="ExternalInput").ap()
    sm_d = nc.dram_tensor("smp", [nblocks, 128, 6 * F], bf16,
                          kind="ExternalInput").ap()
    id_d = nc.dram_tensor("ident", [128, 256], bf16,
                          kind="ExternalInput").ap()
    out_d = nc.dram_tensor("out", [128, 4], f32, kind="ExternalOutput").ap()

    with tile.TileContext(nc) as tc, ExitStack() as ctx:
        inp = ctx.enter_context(tc.tile_pool(name="inp", bufs=2))
        wk = ctx.enter_context(tc.tile_pool(name="wk", bufs=2))
        one = ctx.enter_context(tc.tile_pool(name="one", bufs=1))
        psum = ctx.enter_context(tc.tile_pool(name="psum", bufs=2,
                                              space="PSUM"))

        pew = one.tile([128, 256], bf16, name="pew", tag="pew")
        W1 = pew[:, 0:128]    # I
        W2 = pew[:, 128:256]  # 2I

        NE = nblocks
        zs = one.tile([128, NE], f32, name="zs", tag="zs")    # sum t1+t3
        t2s = one.tile([128, NE], f32, name="t2s", tag="t2s")  # sum ln detN
        out_t = one.tile([128, 4], f32, name="out_t", tag="out_t")
        zjunk = one.tile([128, 2 * F], bf16, name="zjunk", tag="zjunk")

        def bc(view, shape):
            return view.to_broadcast(shape)

        prev = None  # (det2, q2, LL, rr, ecol) of previous block

        for i in range(nblocks):
            in_m = inp.tile([128, 6 * F], bf16, name="in_m", tag="in_m")
            nc.sync.dma_start(out=in_m[:], in_=dmu_d[i])
            in_s = inp.tile([128, 14 * F], bf16, name="in_s", tag="in_s")
            nc.scalar.dma_start(out=in_s[:, 0:7 * F],
                                in_=s_d[i].rearrange(
                                    "p (h n) -> p h n", h=2)[:, 0, :])
            nc.gpsimd.dma_start(out=in_s[:, 7 * F:14 * F],
                                in_=s_d[i].rearrange(
                                    "p (h n) -> p h n", h=2)[:, 1, :])
            VV = inp.tile([128, 12 * F], bf16, name="vv", tag="vv")
            nc.scalar.dma_start(out=VV[:, 6 * F:12 * F], in_=sm_d[i])
            if i == 0:
                nc.sync.dma_start(out=pew, in_=id_d)

            # S view: [p, g(Y|N), k(7 planes), n]
            Sv = in_s.rearrange("p (g k n) -> p g k n", g=2, k=7)
            M12 = wk.tile([128, 24 * F], bf16, name="m12", tag="m12")
            Mv = M12.rearrange("p (g s n) -> p g s n", g=2, s=12)
            CF = wk.tile([128, 12 * F], bf16, name="cf", tag="cf")
            CFv = CF.rearrange("p (g s n) -> p g s n", g=2, s=6)
            W = wk.tile([128, 6 * F], bf16, name="w", tag="w")
            Wv = W.rearrange("p (g s n) -> p g s n", g=2, s=3)
            D3 = wk.tile([128, 3 * F], bf16, name="d3", tag="d3")
            D3k = D3.rearrange("p (k n) -> p k n", k=3)
            VVs = VV[:, 0:6 * F].rearrange("p (s n) -> p s n", s=6)
            QU = wk.tile([128, 12 * F], bf16, name="qu", tag="qu")

            # ---- DVE ----
            # d = t - m  (planes [d2,d1,d0]); dmu is its own small DMA
            nc.vector.tensor_tensor(
                D3[:], in_m[:, 0:3 * F], in_m[:, 3 * F:6 * F], OP.subtract)
            # D6: d0*[d1,d2] -> VV slots 3,1 ; d1*d2 -> slot 2 (early:
            # only depends on dmu, fills the S-DMA wait on block 0)
            nc.vector.tensor_tensor(
                VVs[:, 3:0:-2, :], bc(D3k[:, 2:3, :], (128, 2, F)),
                D3k[:, 1::-1, :], OP.mult)
            nc.vector.tensor_tensor(
                VVs[:, 2:3, :], D3k[:, 1:2, :], D3k[:, 0:1, :], OP.mult)
            # products; block 0 runs per-matrix so Y starts before SN lands
            for gs in ([slice(0, 1), slice(1, 2)] if i == 0
                       else [slice(0, 2)]):
                ng = gs.stop - gs.start
                Sg = Sv[:, gs]
                Mg = Mv[:, gs]
                # Pa: b*[f,c] -> M1[1]=bf, M1[2]=bc
                nc.vector.tensor_tensor(
                    Mg[:, :, 1:3, :], bc(Sg[:, :, 5:6, :], (128, ng, 2, F)),
                    Sg[:, :, 1::-1, :], OP.mult)
                # Pb: [b,c]*[i,e] -> M2[3]=bi (slot 9), M2[1]=ce (slot 7)
                nc.vector.tensor_tensor(
                    Mg[:, :, 9:6:-2, :], Sg[:, :, 5::-5, :],
                    Sg[:, :, 2:4, :], OP.mult)
                # Pc: a*[e,i,f] -> M1[0]=ae, M1[4]=ai, M2[2]=af
                nc.vector.tensor_tensor(
                    Mg[:, :, 0:12:4, :], bc(Sg[:, :, 4:5, :],
                                            (128, ng, 3, F)),
                    Sg[:, :, 3:0:-1, :], OP.mult)
                # Pde: [c,e]*[f,i] -> M1[3]=cf, M1[5]=ei (slots 3,5)
                nc.vector.tensor_tensor(
                    Mg[:, :, 3:7:2, :], Sg[:, :, 0:4:3, :],
                    Sg[:, :, 1:3, :], OP.mult)
            # ---- ACT (emitted before their DVE consumers CF/QU) ----
            # M2 squares: [c^2,f^2] -> slots 10,11 ; b^2 -> slot 6
            nc.scalar.activation(Mv[:, :, 10:12, :], Sv[:, :, 0:2, :],
                                 AF.Square)
            nc.scalar.activation(Mv[:, :, 6:7, :], Sv[:, :, 5:6, :],
                                 AF.Square)
            # D6 squares: (d2,d1) -> VV slots 0,4 ; d0 -> slot 5
            nc.scalar.activation(VVs[:, 0:5:4, :], D3k[:, 0:2, :],
                                 AF.Square)
            nc.scalar.activation(VVs[:, 5:6, :], D3k[:, 2:3, :],
                                 AF.Square)

            # CF = M1 - M2
            nc.vector.tensor_tensor(
                CF[:], Mv[:, :, 0:6, :], Mv[:, :, 6:12, :], OP.subtract)
            # W: [C00,C01,C02]*[a,b,c] (CF slots 5,3,1 ; S slots 4,5,6)
            nc.vector.tensor_tensor(
                Wv[:, :, 0:3, :], CFv[:, :, 5:0:-2, :],
                Sv[:, :, 4:7, :], OP.mult)
            # QU: [Q6|U6] = CF o [D6|SM']
            nc.vector.tensor_tensor(QU[:], CF[:], VV[:], OP.mult)

            # ---- previous block's z (only DVE-consumed tail is deferred) --
            if prev is not None:
                emit_z(nc, prev, zs, zjunk, F)

            det2 = psum.tile([128, 2 * F], f32, name="det2", tag="det2")
            q2 = psum.tile([128, 2 * F], f32, name="q2", tag="q2")
            LL = wk.tile([128, 2 * F], f32, name="ll", tag="ll")
            rr = wk.tile([128, 2 * F], f32, name="rr", tag="rr")

            # PE: det chains (weights I), then u/q chains (same block)
            for g in range(2):
                for j in range(3):
                    nc.tensor.matmul(det2[:, g * F:(g + 1) * F], W1,
                                     Wv[:, g, j, :],
                                     start=(j == 0), stop=(j == 2))
            QUv = QU.rearrange("p (g s n) -> p g s n", g=2, s=6)
            for j in range(6):
                nc.tensor.matmul(q2[:, F:2 * F], W1, QUv[:, 1, j, :],
                                 start=(j == 0), stop=(j == 5))
            # q chain: I on diag slots (0,4,5), 2I on off-diag (1,2,3)
            qorder = [(0, W1, True, False), (4, W1, False, False),
                      (5, W1, False, False), (1, W2, False, False),
                      (2, W2, False, False), (3, W2, False, True)]
            for s, wgt, st, sp in qorder:
                nc.tensor.matmul(q2[:, 0:F], wgt, QUv[:, 0, s, :],
                                 start=st, stop=sp)
            # ACT: logs + reciprocals (same block; ACT has slack)
            nc.scalar.activation(LL[:, 0:F], det2[:, 0:F], AF.Ln)
            nc.scalar.activation(LL[:, F:2 * F], det2[:, F:2 * F], AF.Ln,
                                 accum_out=t2s[:, i:i + 1])
            nc.scalar.activation(rr[:], LL[:], AF.Exp, scale=-1.0)

            prev = (q2, rr, i)

        emit_z(nc, prev, zs, zjunk, F)
        nc.vector.reduce_sum(out_t[:, 0:1], zs[:], axis=AX.X)
        nc.vector.reduce_sum(out_t[:, 1:2], t2s[:], axis=AX.X)
        nc.vector.reduce_sum(out_t[:, 2:3], t2s[:, 0:1], axis=AX.X)
        nc.vector.reduce_sum(out_t[:, 3:4], zs[:, 0:1], axis=AX.X)
        nc.sync.dma_start(out=out_d, in_=out_t[:])

    nc.compile()
    return nc


def emit_z(nc, prev, zs, zjunk, F):
    """z = (q2 * 0.5) * (1/det), accumulated -> sum(t1)+sum(t3)."""
    q2, rr, ecol = prev
    nc.vector.scalar_tensor_tensor(
        zjunk[:], q2[:], 0.5, rr[:], OP.mult, OP.mult,
        accum_out=zs[:, ecol:ecol + 1])


_CACHE = {}


def get_nc(nblocks=4, ncols=512):
    key = (nblocks, ncols)
    if key not in _CACHE:
        _CACHE[key] = build(nblocks, ncols)
    return _CACHE[key]


def make_pew():
    import ml_dtypes

    eye = np.eye(128, dtype=np.float32)
    return np.concatenate([eye, 2.0 * eye], axis=1).astype(ml_dtypes.bfloat16)


def make_in_maps(target, mu, sigma_mu, sigma_n, sigma_y):
    import ml_dtypes

    bf = ml_dtypes.bfloat16
    Bb, C, M, N = target.shape
    nb = M // 128
    F = N
    pew = make_pew()
    in_maps = []
    for b in range(Bb):
        sy = np.asarray(sigma_y[b], np.float32).reshape(M * N, 9)
        sn = np.asarray(sigma_n[b], np.float32).reshape(M * N, 9)
        sm = np.asarray(sigma_mu[b], np.float32).reshape(M * N, 9)
        dmu = np.empty((6, M, N), np.float32)
        dmu[0:3] = np.asarray(target[b], np.float32)[DIDX]
        dmu[3:6] = np.asarray(mu[b], np.float32)[DIDX]
        dmu_p = np.ascontiguousarray(
            dmu.reshape(6, nb, 128, F).transpose(1, 2, 0, 3)
            .reshape(nb, 128, 6 * F).astype(bf))
        sig = np.empty((14, M, N), np.float32)
        sig[0:7] = sy[:, SIDX].T.reshape(7, M, N)
        sig[7:14] = sn[:, SIDX].T.reshape(7, M, N)
        sig_p = np.ascontiguousarray(
            sig.reshape(14, nb, 128, F).transpose(1, 2, 0, 3)
            .reshape(nb, 128, 14 * F).astype(bf))
        smp = (sm[:, MIDX] * MW).T.reshape(6, M, N)
        smp = np.ascontiguousarray(
            smp.reshape(6, nb, 128, F).transpose(1, 2, 0, 3)
            .reshape(nb, 128, 6 * F).astype(bf))
        in_maps.append({"dmu": dmu_p, "sig": sig_p, "smp": smp,
                        "ident": pew})
    return in_maps


def combine(results, n_pixels):
    zsum = 0.0
    t2 = 0.0
    for r in results:
        o = np.asarray(r["out"], dtype=np.float64)
        zsum += o[:, 0].sum()
        t2 += o[:, 1].sum()
    loss = (zsum + 0.5 * t2) / n_pixels
    return np.float32(loss)


def kernel(target, mu, sigma_mu, sigma_n, sigma_y):
    target = np.asarray(target)
    nb = target.shape[2] // 128
    nc = get_nc(nb, target.shape[3])
    in_maps = make_in_maps(target, mu, sigma_mu, sigma_n, sigma_y)
    res = run_bass_kernel_spmd(nc, in_maps, list(range(len(in_maps))))
    n_pixels = target.shape[0] * target.shape[2] * target.shape[3]
    return combine(res.results, n_pixels)


def run_traced(target, mu, sigma_mu, sigma_n, sigma_y, **trace_kwargs):
    target = np.asarray(target)
    nb = target.shape[2] // 128
    nc = get_nc(nb, target.shape[3])
    in_maps = make_in_maps(target, mu, sigma_mu, sigma_n, sigma_y)
    res = run_bass_kernel_spmd(
        nc, in_maps, list(range(len(in_maps))), trace=True, **trace_kwargs)
    n_pixels = target.shape[0] * target.shape[2] * target.shape[3]
    return combine(res.results, n_pixels), res
